# revision 12
# baseline (speedup 1.0000x reference)
"""DeepSeek-V3.1 MoE block (B=2,S=512,H=1024,I=512,E=64,topK=8) on 8 trn2 cores.

Strategy (expert-parallel, sparse dispatch):
  - The reference's dense-masked MoE is mathematically top-8 sparse: only the
    top-8 experts per token contribute (mask is 0 elsewhere). We exploit that.
  - Host: router (fp64 numpy, selection margin on this regime is ~4e-6 >>
    rounding noise), top-8 per token, per-expert token gather with capacity
    padding (C = multiple of 64, >= max load).
  - Device, per core c: experts 8c..8c+7. For each expert, stream
    gate/up/down weights from HBM (fp32r), compute
    Y_e = (silu(X_e @ Wg) * (X_e @ Wu) * w_route) @ Wd on the PE in fp32r
    (tf32-class precision, full PE rate at free-dim >= 256).
    Shared expert: token-parallel (each core takes 128 of the 1024 tokens).
  - Host: scatter-add per-expert outputs back by token, add shared.

Everything is DMA-bound by weight streaming (~50 MB fp32/core, used once).
"""
import sys
sys.path.insert(0, '/opt/trn_rl_repo')
import numpy as np

B, S, H, I, E, TOPK = 2, 512, 1024, 512, 64, 8
T = B * S
NCORES = 8
ELOC = E // NCORES
HC, IC = H // 128, I // 128
TSH = T // NCORES  # shared-expert tokens per core (128)

LAST_RESULT = None  # BassKernelResults of the most recent run (for test harness)


def _pmajor(a, nchunk):
    """[nchunk*128, F] -> partition-major [128, nchunk*F] (chunk-row-major)."""
    F = a.shape[1]
    return np.ascontiguousarray(
        a.reshape(nchunk, 128, F).transpose(1, 0, 2).reshape(128, nchunk * F))


def _build(C):
    import concourse.bacc as bacc
    import concourse.mybir as mybir
    from concourse import tile, masks

    F32 = mybir.dt.float32
    F32R = mybir.dt.float32r
    SILU = mybir.ActivationFunctionType.Silu

    blocks = [(r0, min(128, C - r0)) for r0 in range(0, C, 128)]
    NBLK = len(blocks)

    nc = bacc.Bacc("TRN2", target_bir_lowering=False, debug=False)

    xg_d = nc.dram_tensor("xg", [ELOC, 128, HC * C], F32R, kind="ExternalInput")
    wg_d = nc.dram_tensor("wg", [ELOC, 128, HC * I], F32R, kind="ExternalInput")
    wu_d = nc.dram_tensor("wu", [ELOC, 128, HC * I], F32R, kind="ExternalInput")
    wd_d = nc.dram_tensor("wd", [ELOC, 128, IC * H], F32R, kind="ExternalInput")
    cf_d = nc.dram_tensor("cf", [128, ELOC * NBLK], F32, kind="ExternalInput")
    xs_d = nc.dram_tensor("xs", [128, HC * TSH], F32R, kind="ExternalInput")
    wgs_d = nc.dram_tensor("wgs", [128, HC * I], F32R, kind="ExternalInput")
    wus_d = nc.dram_tensor("wus", [128, HC * I], F32R, kind="ExternalInput")
    wds_d = nc.dram_tensor("wds", [128, IC * H], F32R, kind="ExternalInput")
    yg_d = nc.dram_tensor("yg", [ELOC, C, H], F32, kind="ExternalOutput")
    ys_d = nc.dram_tensor("ys", [TSH, H], F32, kind="ExternalOutput")

    with tile.TileContext(nc) as tc:
        with (
            tc.tile_pool(name="const", bufs=1) as cpool,
            tc.tile_pool(name="wp", bufs=3) as wpool,
            tc.tile_pool(name="xp", bufs=3) as xpool,
            tc.tile_pool(name="ap", bufs=3) as apool,
            tc.tile_pool(name="ps", bufs=2, space="PSUM") as pspool,
        ):
            ident = cpool.tile([128, 128], F32)
            masks.make_identity(nc, ident[:])

            cf_all = cpool.tile([128, ELOC * NBLK], mybir.dt.float32)
            nc.sync.dma_start(cf_all[:], cf_d[:])

            def ffn_block(xg_t, wg_t, wu_t, wd_t, rows, r0, C_in, coef_ap, out_ap):
                """One <=128-row block through SwiGLU + down-proj.

                xg_t: [128, HC*C_in] X^T; weights partition-major; coef_ap
                [rows,1] routing weight per row (or None); out_ap DRAM [rows,H].
                """
                g_ps = pspool.tile([128, I], mybir.dt.float32, tag="g")
                u_ps = pspool.tile([128, I], mybir.dt.float32, tag="u")
                for h in range(HC):
                    nc.tensor.matmul(g_ps[:rows], xg_t[:, h * C_in + r0:h * C_in + r0 + rows],
                                     wg_t[:, h * I:(h + 1) * I],
                                     start=(h == 0), stop=(h == HC - 1))
                for h in range(HC):
                    nc.tensor.matmul(u_ps[:rows], xg_t[:, h * C_in + r0:h * C_in + r0 + rows],
                                     wu_t[:, h * I:(h + 1) * I],
                                     start=(h == 0), stop=(h == HC - 1))
                a_sb = apool.tile([128, I], mybir.dt.float32, tag="a")
                nc.scalar.activation(a_sb[:rows], g_ps[:rows], SILU)
                nc.vector.tensor_mul(a_sb[:rows], a_sb[:rows], u_ps[:rows])
                if coef_ap is not None:
                    nc.vector.tensor_scalar_mul(a_sb[:rows], a_sb[:rows], coef_ap)
                at_sb = apool.tile([128, IC * 128], F32R, tag="at")
                for i in range(IC):
                    t_ps = pspool.tile([128, 128], mybir.dt.float32, tag="t")
                    nc.tensor.transpose(t_ps[:, :rows],
                                        a_sb[:rows, i * 128:(i + 1) * 128],
                                        ident[:rows, :rows])
                    nc.vector.tensor_copy(at_sb[:, i * 128:i * 128 + rows], t_ps[:, :rows])
                y_sb = apool.tile([128, H], mybir.dt.float32, tag="ysb")
                for half in range(2):
                    y_ps = pspool.tile([128, 512], mybir.dt.float32, tag="y")
                    for i in range(IC):
                        nc.tensor.matmul(y_ps[:rows], at_sb[:, i * 128:i * 128 + rows],
                                         wd_t[:, i * H + 512 * half:i * H + 512 * (half + 1)],
                                         start=(i == 0), stop=(i == IC - 1))
                    nc.vector.tensor_copy(y_sb[:rows, 512 * half:512 * (half + 1)], y_ps[:rows])
                nc.gpsimd.dma_start(out_ap, y_sb[:rows])

            for e in range(ELOC):
                wg_t = wpool.tile([128, HC * I], F32R, tag="wg")
                wu_t = wpool.tile([128, HC * I], F32R, tag="wu")
                wd_t = wpool.tile([128, IC * H], F32R, tag="wd")
                xg_t = xpool.tile([128, HC * C], F32R, tag="xg")
                nc.sync.dma_start(xg_t[:], xg_d[e])
                hh = HC * I // 2
                nc.sync.dma_start(wg_t[:, :hh], wg_d[e][:, :hh])
                nc.sync.dma_start(wg_t[:, hh:], wg_d[e][:, hh:])
                nc.sync.dma_start(wu_t[:, :hh], wu_d[e][:, :hh])
                nc.sync.dma_start(wu_t[:, hh:], wu_d[e][:, hh:])
                ih = IC * H // 2
                nc.sync.dma_start(wd_t[:, :ih], wd_d[e][:, :ih])
                nc.sync.dma_start(wd_t[:, ih:], wd_d[e][:, ih:])
                for b, (r0, rows) in enumerate(blocks):
                    ffn_block(xg_t, wg_t, wu_t, wd_t, rows, r0, C,
                              cf_all[:rows, e * NBLK + b:e * NBLK + b + 1],
                              yg_d[e, r0:r0 + rows, :])

            # shared expert on this core's token slice
            wgs_t = wpool.tile([128, HC * I], F32R, tag="wg")
            wus_t = wpool.tile([128, HC * I], F32R, tag="wu")
            wds_t = wpool.tile([128, IC * H], F32R, tag="wd")
            xs_t = xpool.tile([128, HC * TSH], F32R, tag="xg")
            nc.sync.dma_start(xs_t[:], xs_d[:])
            nc.sync.dma_start(wgs_t[:], wgs_d[:])
            nc.sync.dma_start(wus_t[:], wus_d[:])
            nc.sync.dma_start(wds_t[:], wds_d[:])
            ffn_block(xs_t, wgs_t, wus_t, wds_t, TSH, 0, TSH, None, ys_d[:, :])

    nc.compile()
    return nc


def prepare(hidden_states, router_w, shared_gate_w, shared_up_w, shared_down_w,
            expert_gate_k, expert_up_k, expert_down_k):
    """Host-side routing + dispatch. Returns (nc, in_maps, meta)."""
    x = np.ascontiguousarray(np.asarray(hidden_states, dtype=np.float32).reshape(T, H))
    rw = np.asarray(router_w, dtype=np.float32)
    egk = np.asarray(expert_gate_k, dtype=np.float32)
    euk = np.asarray(expert_up_k, dtype=np.float32)
    edk = np.asarray(expert_down_k, dtype=np.float32)
    sgw = np.asarray(shared_gate_w, dtype=np.float32)
    suw = np.asarray(shared_up_w, dtype=np.float32)
    sdw = np.asarray(shared_down_w, dtype=np.float32)

    # ---- routing on host (fp64; selection margin >> fp32 noise) ----
    logits = x.astype(np.float64) @ rw.astype(np.float64)
    aff = 1.0 / (1.0 + np.exp(-logits))
    top_idx = np.argpartition(-aff, TOPK - 1, axis=1)[:, :TOPK]        # [T,8]
    top_vals = np.take_along_axis(aff, top_idx, axis=1)
    top_w = top_vals / (top_vals.sum(axis=1, keepdims=True) + 1e-9)    # [T,8]

    flat_e = top_idx.ravel()
    flat_t = np.repeat(np.arange(T), TOPK)
    flat_w = top_w.ravel()
    order = np.argsort(flat_e, kind="stable")
    se, st, sw = flat_e[order], flat_t[order], flat_w[order]
    counts = np.bincount(flat_e, minlength=E)
    offs = np.concatenate([[0], np.cumsum(counts)])

    C = int(max(128, -(-counts.max() // 32) * 32))
    NBLK = (C + 127) // 128

    nc = _build(C)

    in_maps = []
    for c in range(NCORES):
        xg = np.zeros((ELOC, 128, HC * C), np.float32)
        cf = np.zeros((128, ELOC * NBLK), np.float32)
        for el in range(ELOC):
            e = ELOC * c + el
            toks = st[offs[e]:offs[e + 1]]
            ws = sw[offs[e]:offs[e + 1]]
            n = len(toks)
            xe = np.zeros((C, H), np.float32)
            xe[:n] = x[toks]
            xg[el] = _pmajor(np.ascontiguousarray(xe.T), HC)
            cfp = np.zeros(NBLK * 128, np.float32)
            cfp[:n] = ws
            cf[:, el * NBLK:(el + 1) * NBLK] = cfp.reshape(NBLK, 128).T
        def wstack(w, nchunk):  # [ELOC, nchunk*128, F] -> [ELOC, 128, nchunk*F]
            F = w.shape[2]
            return np.ascontiguousarray(
                w.reshape(ELOC, nchunk, 128, F).transpose(0, 2, 1, 3)
                 .reshape(ELOC, 128, nchunk * F))

        in_maps.append({
            "xg": xg,
            "wg": wstack(egk[ELOC * c:ELOC * (c + 1)], HC),
            "wu": wstack(euk[ELOC * c:ELOC * (c + 1)], HC),
            "wd": wstack(edk[ELOC * c:ELOC * (c + 1)], IC),
            "cf": cf,
            "xs": _pmajor(np.ascontiguousarray(x[TSH * c:TSH * (c + 1)].T), HC),
            "wgs": _pmajor(sgw, HC),
            "wus": _pmajor(suw, HC),
            "wds": _pmajor(sdw, IC),
        })

    return nc, in_maps, (st, offs)


def assemble(results, meta):
    st, offs = meta
    out = np.zeros((T, H), np.float32)
    for c in range(NCORES):
        r = results[c]
        out[TSH * c:TSH * (c + 1)] += r["ys"]
        yg = r["yg"]
        for el in range(ELOC):
            e = ELOC * c + el
            toks = st[offs[e]:offs[e + 1]]
            out[toks] += yg[el, :len(toks)]
    return out.reshape(B, S, H)


def kernel(**inputs):
    global LAST_RESULT
    import os, time
    from concourse.bass_utils import run_bass_kernel_spmd
    if os.environ.get("BASS_TRACE"):
        try:
            import antenv.axon_hooks  # noqa: F401
        except ImportError:
            # trace requested but the axon NTFF hook module isn't present in
            # this container -- tracing would crash mid-run; disable it.
            os.environ["BASS_NEVER_TRACE"] = "1"
    nc, in_maps, meta = prepare(**inputs)
    last_err = None
    for attempt in range(3):
        try:
            res = run_bass_kernel_spmd(nc, in_maps, core_ids=list(range(NCORES)))
            break
        except Exception as err:  # transient device faults (e.g. NRT exec errors)
            last_err = err
            time.sleep(5 * (attempt + 1))
    else:
        raise last_err
    LAST_RESULT = res
    return assemble(res.results, meta)


# revision 14
# speedup vs baseline: 1.0651x; 1.0651x over previous
"""DeepSeek-V3.1 MoE block (B=2,S=512,H=1024,I=512,E=64,topK=8) on 8 trn2 cores.

Strategy (expert-parallel, sparse dispatch):
  - The reference's dense-masked MoE is mathematically top-8 sparse: only the
    top-8 experts per token contribute (mask is 0 elsewhere). We exploit that.
  - Host: router (fp64 numpy, selection margin on this regime is ~4e-6 >>
    rounding noise), top-8 per token, per-expert token gather with capacity
    padding (C = multiple of 32, >= max per-expert load).
  - Device, per core c: experts 8c..8c+7. For each expert, stream
    gate/up/down weights from HBM (fp32r), compute
    Y_e = (silu(X_e @ Wg) * (X_e @ Wu) * w_route) @ Wd on the PE in fp32r
    (tf32-class precision, full PE rate at free-dim >= 256).
    Shared expert: token-parallel (each core takes 128 of the 1024 tokens).
  - Host: scatter-add per-expert outputs back by token, add shared.

Everything is DMA-bound by weight streaming (~50 MB fp32/core, used once).
"""
import os as _os, sys
try:
    import concourse  # noqa: F401  (env-provided, e.g. axon boot path)
except ImportError:
    for _p in ('/root/.axon_site/_ro/trn_rl_repo', '/opt/trn_rl_repo'):
        if _os.path.isdir(_p) and _p not in sys.path:
            sys.path.append(_p)
import numpy as np

B, S, H, I, E, TOPK = 2, 512, 1024, 512, 64, 8
T = B * S
NCORES = 8
ELOC = E // NCORES
HC, IC = H // 128, I // 128
TSH = T // NCORES  # shared-expert tokens per core (128)

LAST_RESULT = None  # BassKernelResults of the most recent run (for test harness)


def _pmajor(a, nchunk):
    """[nchunk*128, F] -> partition-major [128, nchunk*F] (chunk-row-major)."""
    F = a.shape[1]
    return np.ascontiguousarray(
        a.reshape(nchunk, 128, F).transpose(1, 0, 2).reshape(128, nchunk * F))


def _build(C):
    import concourse.bacc as bacc
    import concourse.mybir as mybir
    from concourse import tile, masks

    F32 = mybir.dt.float32
    F32R = mybir.dt.float32r
    SILU = mybir.ActivationFunctionType.Silu

    blocks = [(r0, min(128, C - r0)) for r0 in range(0, C, 128)]
    NBLK = len(blocks)

    nc = bacc.Bacc("TRN2", target_bir_lowering=False, debug=False)

    xg_d = nc.dram_tensor("xg", [ELOC, 128, HC * C], F32R, kind="ExternalInput")
    wg_d = nc.dram_tensor("wg", [ELOC, 128, HC * I], F32R, kind="ExternalInput")
    wu_d = nc.dram_tensor("wu", [ELOC, 128, HC * I], F32R, kind="ExternalInput")
    wd_d = nc.dram_tensor("wd", [ELOC, 128, IC * H], F32R, kind="ExternalInput")
    cf_d = nc.dram_tensor("cf", [128, ELOC * NBLK], F32, kind="ExternalInput")
    xs_d = nc.dram_tensor("xs", [128, HC * TSH], F32R, kind="ExternalInput")
    wgs_d = nc.dram_tensor("wgs", [128, HC * I], F32R, kind="ExternalInput")
    wus_d = nc.dram_tensor("wus", [128, HC * I], F32R, kind="ExternalInput")
    wds_d = nc.dram_tensor("wds", [128, IC * H], F32R, kind="ExternalInput")
    yg_d = nc.dram_tensor("yg", [ELOC, C, H], F32, kind="ExternalOutput")
    ys_d = nc.dram_tensor("ys", [TSH, H], F32, kind="ExternalOutput")

    with tile.TileContext(nc) as tc:
        with (
            tc.tile_pool(name="const", bufs=1) as cpool,
            tc.tile_pool(name="wp", bufs=3) as wpool,
            tc.tile_pool(name="xp", bufs=3) as xpool,
            tc.tile_pool(name="ap", bufs=3) as apool,
            tc.tile_pool(name="ps", bufs=2, space="PSUM") as pspool,
        ):
            ident = cpool.tile([128, 128], F32)
            masks.make_identity(nc, ident[:])

            cf_all = cpool.tile([128, ELOC * NBLK], mybir.dt.float32)
            nc.sync.dma_start(cf_all[:], cf_d[:])

            def ffn_block(xg_t, wg_t, wu_t, wd_t, rows, r0, C_in, coef_ap, out_ap):
                """One <=128-row block through SwiGLU + down-proj.

                xg_t: [128, HC*C_in] X^T; weights partition-major; coef_ap
                [rows,1] routing weight per row (or None); out_ap DRAM [rows,H].
                """
                g_ps = pspool.tile([128, I], mybir.dt.float32, tag="g")
                u_ps = pspool.tile([128, I], mybir.dt.float32, tag="u")
                for h in range(HC):
                    nc.tensor.matmul(g_ps[:rows], xg_t[:, h * C_in + r0:h * C_in + r0 + rows],
                                     wg_t[:, h * I:(h + 1) * I],
                                     start=(h == 0), stop=(h == HC - 1))
                for h in range(HC):
                    nc.tensor.matmul(u_ps[:rows], xg_t[:, h * C_in + r0:h * C_in + r0 + rows],
                                     wu_t[:, h * I:(h + 1) * I],
                                     start=(h == 0), stop=(h == HC - 1))
                a_sb = apool.tile([128, I], mybir.dt.float32, tag="a")
                nc.scalar.activation(a_sb[:rows], g_ps[:rows], SILU)
                nc.vector.tensor_mul(a_sb[:rows], a_sb[:rows], u_ps[:rows])
                if coef_ap is not None:
                    nc.vector.tensor_scalar_mul(a_sb[:rows], a_sb[:rows], coef_ap)
                at_sb = apool.tile([128, IC * 128], F32R, tag="at")
                for i in range(IC):
                    t_ps = pspool.tile([128, 128], mybir.dt.float32, tag="t")
                    nc.tensor.transpose(t_ps[:, :rows],
                                        a_sb[:rows, i * 128:(i + 1) * 128],
                                        ident[:rows, :rows])
                    nc.vector.tensor_copy(at_sb[:, i * 128:i * 128 + rows], t_ps[:, :rows])
                y_sb = apool.tile([128, H], mybir.dt.float32, tag="ysb")
                for half in range(2):
                    y_ps = pspool.tile([128, 512], mybir.dt.float32, tag="y")
                    for i in range(IC):
                        nc.tensor.matmul(y_ps[:rows], at_sb[:, i * 128:i * 128 + rows],
                                         wd_t[:, i * H + 512 * half:i * H + 512 * (half + 1)],
                                         start=(i == 0), stop=(i == IC - 1))
                    nc.vector.tensor_copy(y_sb[:rows, 512 * half:512 * (half + 1)], y_ps[:rows])
                nc.gpsimd.dma_start(out_ap, y_sb[:rows])

            for e in range(ELOC):
                wg_t = wpool.tile([128, HC * I], F32R, tag="wg")
                wu_t = wpool.tile([128, HC * I], F32R, tag="wu")
                wd_t = wpool.tile([128, IC * H], F32R, tag="wd")
                xg_t = xpool.tile([128, HC * C], F32R, tag="xg")
                nc.sync.dma_start(xg_t[:], xg_d[e])
                hh = HC * I // 2
                nc.sync.dma_start(wg_t[:, :hh], wg_d[e][:, :hh])
                nc.sync.dma_start(wg_t[:, hh:], wg_d[e][:, hh:])
                nc.sync.dma_start(wu_t[:, :hh], wu_d[e][:, :hh])
                nc.sync.dma_start(wu_t[:, hh:], wu_d[e][:, hh:])
                ih = IC * H // 2
                nc.sync.dma_start(wd_t[:, :ih], wd_d[e][:, :ih])
                nc.sync.dma_start(wd_t[:, ih:], wd_d[e][:, ih:])
                for b, (r0, rows) in enumerate(blocks):
                    ffn_block(xg_t, wg_t, wu_t, wd_t, rows, r0, C,
                              cf_all[:rows, e * NBLK + b:e * NBLK + b + 1],
                              yg_d[e, r0:r0 + rows, :])

            # shared expert on this core's token slice
            wgs_t = wpool.tile([128, HC * I], F32R, tag="wg")
            wus_t = wpool.tile([128, HC * I], F32R, tag="wu")
            wds_t = wpool.tile([128, IC * H], F32R, tag="wd")
            xs_t = xpool.tile([128, HC * TSH], F32R, tag="xg")
            nc.sync.dma_start(xs_t[:], xs_d[:])
            nc.sync.dma_start(wgs_t[:], wgs_d[:])
            nc.sync.dma_start(wus_t[:], wus_d[:])
            nc.sync.dma_start(wds_t[:], wds_d[:])
            ffn_block(xs_t, wgs_t, wus_t, wds_t, TSH, 0, TSH, None, ys_d[:, :])

    nc.compile()
    return nc


def prepare(hidden_states, router_w, shared_gate_w, shared_up_w, shared_down_w,
            expert_gate_k, expert_up_k, expert_down_k):
    """Host-side routing + dispatch. Returns (nc, in_maps, meta)."""
    x = np.ascontiguousarray(np.asarray(hidden_states, dtype=np.float32).reshape(T, H))
    rw = np.asarray(router_w, dtype=np.float32)
    egk = np.asarray(expert_gate_k, dtype=np.float32)
    euk = np.asarray(expert_up_k, dtype=np.float32)
    edk = np.asarray(expert_down_k, dtype=np.float32)
    sgw = np.asarray(shared_gate_w, dtype=np.float32)
    suw = np.asarray(shared_up_w, dtype=np.float32)
    sdw = np.asarray(shared_down_w, dtype=np.float32)

    # ---- routing on host (fp64; selection margin >> fp32 noise) ----
    logits = x.astype(np.float64) @ rw.astype(np.float64)
    aff = 1.0 / (1.0 + np.exp(-logits))
    top_idx = np.argpartition(-aff, TOPK - 1, axis=1)[:, :TOPK]        # [T,8]
    top_vals = np.take_along_axis(aff, top_idx, axis=1)
    top_w = top_vals / (top_vals.sum(axis=1, keepdims=True) + 1e-9)    # [T,8]

    flat_e = top_idx.ravel()
    flat_t = np.repeat(np.arange(T), TOPK)
    flat_w = top_w.ravel()
    order = np.argsort(flat_e, kind="stable")
    se, st, sw = flat_e[order], flat_t[order], flat_w[order]
    counts = np.bincount(flat_e, minlength=E)
    offs = np.concatenate([[0], np.cumsum(counts)])

    C = int(max(128, -(-counts.max() // 32) * 32))
    NBLK = (C + 127) // 128

    nc = _build(C)

    in_maps = []
    for c in range(NCORES):
        xg = np.zeros((ELOC, 128, HC * C), np.float32)
        cf = np.zeros((128, ELOC * NBLK), np.float32)
        for el in range(ELOC):
            e = ELOC * c + el
            toks = st[offs[e]:offs[e + 1]]
            ws = sw[offs[e]:offs[e + 1]]
            n = len(toks)
            xe = np.zeros((C, H), np.float32)
            xe[:n] = x[toks]
            xg[el] = _pmajor(np.ascontiguousarray(xe.T), HC)
            cfp = np.zeros(NBLK * 128, np.float32)
            cfp[:n] = ws
            cf[:, el * NBLK:(el + 1) * NBLK] = cfp.reshape(NBLK, 128).T
        def wstack(w, nchunk):  # [ELOC, nchunk*128, F] -> [ELOC, 128, nchunk*F]
            F = w.shape[2]
            return np.ascontiguousarray(
                w.reshape(ELOC, nchunk, 128, F).transpose(0, 2, 1, 3)
                 .reshape(ELOC, 128, nchunk * F))

        in_maps.append({
            "xg": xg,
            "wg": wstack(egk[ELOC * c:ELOC * (c + 1)], HC),
            "wu": wstack(euk[ELOC * c:ELOC * (c + 1)], HC),
            "wd": wstack(edk[ELOC * c:ELOC * (c + 1)], IC),
            "cf": cf,
            "xs": _pmajor(np.ascontiguousarray(x[TSH * c:TSH * (c + 1)].T), HC),
            "wgs": _pmajor(sgw, HC),
            "wus": _pmajor(suw, HC),
            "wds": _pmajor(sdw, IC),
        })

    return nc, in_maps, (st, offs)


def assemble(results, meta):
    st, offs = meta
    out = np.zeros((T, H), np.float32)
    for c in range(NCORES):
        r = results[c]
        out[TSH * c:TSH * (c + 1)] += r["ys"]
        yg = r["yg"]
        for el in range(ELOC):
            e = ELOC * c + el
            toks = st[offs[e]:offs[e + 1]]
            out[toks] += yg[el, :len(toks)]
    return out.reshape(B, S, H)


def kernel(**inputs):
    global LAST_RESULT
    import os, time
    from concourse.bass_utils import run_bass_kernel_spmd
    if os.environ.get("BASS_TRACE"):
        try:
            import antenv.axon_hooks  # noqa: F401
        except ImportError:
            # trace requested but the axon NTFF hook module isn't present in
            # this container -- tracing would crash mid-run; disable it.
            os.environ["BASS_NEVER_TRACE"] = "1"
    nc, in_maps, meta = prepare(**inputs)
    last_err = None
    for attempt in range(3):
        try:
            res = run_bass_kernel_spmd(nc, in_maps, core_ids=list(range(NCORES)))
            break
        except Exception as err:  # transient device faults (e.g. NRT exec errors)
            last_err = err
            time.sleep(5 * (attempt + 1))
    else:
        raise last_err
    LAST_RESULT = res
    return assemble(res.results, meta)

